# revision 1
# baseline (speedup 1.0000x reference)
"""LIF (leaky integrate-and-fire) recurrence kernel for Trainium2, 8 NeuronCores.

Problem: x (T=32, B=64, N=32768) f32.
    m[t] = tau*v[t-1] + x[t];  y[t] = (m[t] >= 1.0);  v[t] = m[t]*(1-y[t])
Output: y (32, 64, 32768) f32.

Sharding: data-parallel over batch. Core c handles x[:, 8c:8(c+1), :],
a (32, 262144)-element independent recurrence.

Per-core kernel (bit-exact vs the f32 reference):
  state  w = tau*v  (so m = w + x is one tensor_tensor add)
  per timestep t:
    m = w + x_t                          DVE tensor_tensor add
    g = (m is_lt 1.0) * tau              DVE tensor_scalar (fused two-op)
            g == tau  where m <  1  (no spike)
            g == 0    where m >= 1  (spike, hard reset)
    y_t = Copy(-2*g + 1) = 1 - g/tau     ACT affine (exact: g in {0, 0.5})
    w = m * g                            DVE tensor_tensor mult (= tau*v_new)
Timesteps are chunked 4-at-a-time so DMAs are 4 MiB each.
"""

import sys

if "/opt/trn_rl_repo" not in sys.path:
    sys.path.insert(0, "/opt/trn_rl_repo")

import numpy as np

TAU = 0.5
V_TH = 1.0

N_CORES = 8
T, B, N = 32, 64, 32768
B_SH = B // N_CORES          # 8 batch rows per core
E = B_SH * N                 # 262144 elements per core per timestep
P = 128                      # SBUF partitions
F = E // P                   # 2048 f32 per partition per timestep
TPC = 4                      # timesteps per DMA chunk
C = T // TPC                 # 8 chunks

_compiled = None


def _build():
    from concourse import bacc, tile, mybir

    f32 = mybir.dt.float32
    nc = bacc.Bacc("TRN2", debug=False, num_devices=N_CORES)
    x = nc.dram_tensor("x", [T, E], f32, kind="ExternalInput").ap()
    y = nc.dram_tensor("y", [T, E], f32, kind="ExternalOutput").ap()

    # (c t) rows, (p f) cols -> per-chunk AP [p, t, f]
    x_r = x.rearrange("(c t) (p f) -> c p t f", t=TPC, p=P)
    y_r = y.rearrange("(c t) (p f) -> c p t f", t=TPC, p=P)

    with tile.TileContext(nc) as tc:
        with (
            tc.tile_pool(name="io", bufs=2) as io_pool,
            tc.tile_pool(name="state", bufs=1) as st_pool,
            tc.tile_pool(name="tmp", bufs=2) as tmp_pool,
        ):
            w = st_pool.tile([P, F], f32, tag="w")
            nc.vector.memset(w[:], 0.0)
            for c in range(C):
                x_t = io_pool.tile([P, TPC * F], f32, tag="x")
                y_t = io_pool.tile([P, TPC * F], f32, tag="y")
                nc.sync.dma_start(
                    out=x_t[:].rearrange("p (t f) -> p t f", t=TPC), in_=x_r[c]
                )
                for t in range(TPC):
                    xs = x_t[:, t * F:(t + 1) * F]
                    ys = y_t[:, t * F:(t + 1) * F]
                    m = tmp_pool.tile([P, F], f32, tag="m")
                    g = tmp_pool.tile([P, F], f32, tag="g")
                    nc.vector.tensor_tensor(
                        out=m[:], in0=w[:], in1=xs, op=mybir.AluOpType.add
                    )
                    nc.vector.tensor_scalar(
                        out=g[:], in0=m[:], scalar1=V_TH, scalar2=TAU,
                        op0=mybir.AluOpType.is_lt, op1=mybir.AluOpType.mult,
                    )
                    nc.scalar.activation(
                        out=ys, in_=g[:],
                        func=mybir.ActivationFunctionType.Copy,
                        bias=1.0, scale=-1.0 / TAU,
                    )
                    nc.vector.tensor_tensor(
                        out=w[:], in0=m[:], in1=g[:], op=mybir.AluOpType.mult
                    )
                nc.sync.dma_start(
                    out=y_r[c], in_=y_t[:].rearrange("p (t f) -> p t f", t=TPC)
                )
    nc.compile()
    return nc


def _get_compiled():
    global _compiled
    if _compiled is None:
        _compiled = _build()
    return _compiled


def kernel(x: np.ndarray, _trace: bool = False):
    import concourse.bass_utils as bass_utils

    nc = _get_compiled()
    x = np.ascontiguousarray(x, dtype=np.float32)
    in_maps = [
        {"x": x[:, c * B_SH:(c + 1) * B_SH, :].reshape(T, E)}
        for c in range(N_CORES)
    ]
    res = bass_utils.run_bass_kernel_spmd(
        nc, in_maps, core_ids=list(range(N_CORES)), trace=_trace
    )
    y = np.empty((T, B, N), dtype=np.float32)
    for c in range(N_CORES):
        y[:, c * B_SH:(c + 1) * B_SH, :] = res.results[c]["y"].reshape(T, B_SH, N)
    if _trace:
        return y, res
    return y


# revision 4
# speedup vs baseline: 1.2501x; 1.2501x over previous
"""LIF (leaky integrate-and-fire) recurrence kernel for Trainium2, 8 NeuronCores.

Problem: x (T=32, B=64, N=32768) f32.
    m[t] = tau*v[t-1] + x[t];  y[t] = (m[t] >= 1.0);  v[t] = m[t]*(1-y[t])
Output: y (32, 64, 32768) f32.

Sharding: data-parallel over batch. Core c handles x[:, 8c:8(c+1), :],
a (32, 262144)-element independent recurrence.

Per-core pipeline (bit-exact vs the f32 reference):
  DVE (two fused scalar_tensor_tensor ops per step):
    m = (v * tau) + x_t            (in0 op0 scalar) op1 in1
    v = (m is_lt 1.0) * m          hard reset: v=m below threshold, else 0
  ACT (spike output, exact at the threshold):
    s = Sign(m - 1)                m-1 is exact (Sterbenz), s in {-1,0,+1}
    y = Sigmoid(1e4*s + 5e3)       saturates: s=-1 -> 0.0, s in {0,+1} -> 1.0
                                   (m == 1.0 exactly gives y = 1, as the
                                   reference's u >= 0 does)
Timesteps are chunked 4-at-a-time so DMAs are 4 MiB each; x loads ride the
sync HWDGE ring, y stores the scalar HWDGE ring.
"""

import sys

if "/opt/trn_rl_repo" not in sys.path:
    sys.path.insert(0, "/opt/trn_rl_repo")

import numpy as np

TAU = 0.5
V_TH = 1.0

N_CORES = 8
T, B, N = 32, 64, 32768
B_SH = B // N_CORES          # 8 batch rows per core
E = B_SH * N                 # 262144 elements per core per timestep
P = 128                      # SBUF partitions
F = E // P                   # 2048 f32 per partition per timestep
TPC = 4                      # timesteps per DMA chunk
C = T // TPC                 # 8 chunks

_compiled = None


def _build():
    from concourse import bacc, tile, mybir

    f32 = mybir.dt.float32
    nc = bacc.Bacc("TRN2", debug=False, num_devices=N_CORES)
    x = nc.dram_tensor("x", [T, E], f32, kind="ExternalInput").ap()
    y = nc.dram_tensor("y", [T, E], f32, kind="ExternalOutput").ap()

    # (c t) rows, (p f) cols -> per-chunk AP [p, t, f]
    x_r = x.rearrange("(c t) (p f) -> c p t f", t=TPC, p=P)
    y_r = y.rearrange("(c t) (p f) -> c p t f", t=TPC, p=P)

    with tile.TileContext(nc) as tc:
        with (
            tc.tile_pool(name="io", bufs=2) as io_pool,
            tc.tile_pool(name="state", bufs=1) as st_pool,
            tc.tile_pool(name="m", bufs=3) as m_pool,
            tc.tile_pool(name="s", bufs=2) as s_pool,
        ):
            # per-partition constants for the ACT affine args
            c_neg1 = st_pool.tile([P, 1], f32, tag="c_neg1")
            c_scale = st_pool.tile([P, 1], f32, tag="c_scale")
            c_bias = st_pool.tile([P, 1], f32, tag="c_bias")
            nc.gpsimd.memset(c_neg1[:], -V_TH)
            nc.gpsimd.memset(c_scale[:], 1.0e4)
            nc.gpsimd.memset(c_bias[:], 5.0e3)
            v = st_pool.tile([P, F], f32, tag="v")
            nc.vector.memset(v[:], 0.0)
            for c in range(C):
                x_t = io_pool.tile([P, TPC * F], f32, tag="x")
                y_t = io_pool.tile([P, TPC * F], f32, tag="y")
                nc.sync.dma_start(
                    out=x_t[:].rearrange("p (t f) -> p t f", t=TPC), in_=x_r[c]
                )
                for t in range(TPC):
                    xs = x_t[:, t * F:(t + 1) * F]
                    ys = y_t[:, t * F:(t + 1) * F]
                    m = m_pool.tile([P, F], f32, tag="m")
                    s = s_pool.tile([P, F], f32, tag="s")
                    # m = (v * tau) + x_t
                    nc.vector.scalar_tensor_tensor(
                        out=m[:], in0=v[:], scalar=TAU, in1=xs,
                        op0=mybir.AluOpType.mult, op1=mybir.AluOpType.add,
                    )
                    # v = (m < vth) * m   (hard reset)
                    nc.vector.scalar_tensor_tensor(
                        out=v[:], in0=m[:], scalar=V_TH, in1=m[:],
                        op0=mybir.AluOpType.is_lt, op1=mybir.AluOpType.mult,
                    )
                    # s = Sign(m - 1); y = Sigmoid(1e4*s + 5e3)
                    nc.scalar.activation(
                        out=s[:], in_=m[:],
                        func=mybir.ActivationFunctionType.Sign,
                        bias=c_neg1[:], scale=1.0,
                    )
                    nc.scalar.activation(
                        out=ys, in_=s[:],
                        func=mybir.ActivationFunctionType.Sigmoid,
                        bias=c_bias[:], scale=c_scale[:],
                    )
                nc.scalar.dma_start(
                    out=y_r[c], in_=y_t[:].rearrange("p (t f) -> p t f", t=TPC)
                )
    nc.compile()
    return nc


def _get_compiled():
    global _compiled
    if _compiled is None:
        _compiled = _build()
    return _compiled


def kernel(x: np.ndarray, _trace: bool = False):
    import concourse.bass_utils as bass_utils

    nc = _get_compiled()
    x = np.ascontiguousarray(x, dtype=np.float32)
    in_maps = [
        {"x": x[:, c * B_SH:(c + 1) * B_SH, :].reshape(T, E)}
        for c in range(N_CORES)
    ]
    res = bass_utils.run_bass_kernel_spmd(
        nc, in_maps, core_ids=list(range(N_CORES)), trace=_trace
    )
    y = np.empty((T, B, N), dtype=np.float32)
    for c in range(N_CORES):
        y[:, c * B_SH:(c + 1) * B_SH, :] = res.results[c]["y"].reshape(T, B_SH, N)
    if _trace:
        return y, res
    return y


# revision 7
# speedup vs baseline: 1.2945x; 1.0355x over previous
"""LIF (leaky integrate-and-fire) recurrence kernel for Trainium2, 8 NeuronCores.

Problem: x (T=32, B=64, N=32768) f32.
    m[t] = tau*v[t-1] + x[t];  y[t] = (m[t] >= 1.0);  v[t] = m[t]*(1-y[t])
Output: y (32, 64, 32768) f32.

Sharding: data-parallel over batch. Core c handles x[:, 8c:8(c+1), :],
a (32, 262144)-element independent recurrence.

Per-core pipeline (bit-exact vs the f32 reference):
  DVE (two fused scalar_tensor_tensor ops per step):
    m = (v * tau) + x_t            (in0 op0 scalar) op1 in1
    v = (m is_lt 1.0) * m          hard reset: v=m below threshold, else 0
  ACT (spike output, exact at the threshold):
    s = Sign(m - 1)                m-1 is exact (Sterbenz), s in {-1,0,+1}
    y = Sigmoid(1e4*s + 5e3)       saturates: s=-1 -> 0.0, s in {0,+1} -> 1.0
                                   (m == 1.0 exactly gives y = 1, matching the
                                   reference's u >= 0)
x loads are staged [1,3,4,4,...] timesteps (fast pipeline fill) on the sync
HWDGE ring; y stores flush every 2 timesteps (short drain) on the scalar ring.
"""

import sys

if "/opt/trn_rl_repo" not in sys.path:
    sys.path.insert(0, "/opt/trn_rl_repo")

import numpy as np

TAU = 0.5
V_TH = 1.0

N_CORES = 8
T, B, N = 32, 64, 32768
B_SH = B // N_CORES          # 8 batch rows per core
E = B_SH * N                 # 262144 elements per core per timestep
P = 128                      # SBUF partitions
F = E // P                   # 2048 f32 per partition per timestep

X_CHUNKS = [1, 3] + [4] * 7  # timesteps per x load
Y_CHUNK = 2                  # timesteps per y store

_compiled = None


def _build():
    from concourse import bacc, tile, mybir

    f32 = mybir.dt.float32
    assert sum(X_CHUNKS) == T and T % Y_CHUNK == 0
    nc = bacc.Bacc("TRN2", debug=False, num_devices=N_CORES)
    x = nc.dram_tensor("x", [T, E], f32, kind="ExternalInput").ap()
    y = nc.dram_tensor("y", [T, E], f32, kind="ExternalOutput").ap()

    # [t, p, f] views of DRAM
    x_r = x.rearrange("t (p f) -> t p f", p=P)
    y_r = y.rearrange("t (p f) -> t p f", p=P)

    with tile.TileContext(nc) as tc:
        with (
            tc.tile_pool(name="io", bufs=2) as io_pool,
            tc.tile_pool(name="state", bufs=1) as st_pool,
            tc.tile_pool(name="m", bufs=4) as m_pool,
            tc.tile_pool(name="s", bufs=3) as s_pool,
        ):
            # per-partition constants for the ACT affine args
            c_neg1 = st_pool.tile([P, 1], f32, tag="c_neg1")
            c_scale = st_pool.tile([P, 1], f32, tag="c_scale")
            c_bias = st_pool.tile([P, 1], f32, tag="c_bias")
            nc.gpsimd.memset(c_neg1[:], -V_TH)
            nc.gpsimd.memset(c_scale[:], 1.0e4)
            nc.gpsimd.memset(c_bias[:], 5.0e3)
            v = st_pool.tile([P, F], f32, tag="v")
            nc.gpsimd.memset(v[:], 0.0)

            # issue x loads lazily, two chunks ahead of consumption
            x_tiles = {}          # t -> (tile, col offset)
            next_chunk = 0
            t_loaded = 0

            def load_chunk():
                nonlocal next_chunk, t_loaded
                n_t = X_CHUNKS[next_chunk]
                xt = io_pool.tile([P, 4 * F], f32, tag="x")
                nc.sync.dma_start(
                    out=xt[:, : n_t * F].rearrange("p (t f) -> p t f", t=n_t),
                    in_=x_r[t_loaded:t_loaded + n_t].rearrange("t p f -> p t f"),
                )
                for i in range(n_t):
                    x_tiles[t_loaded + i] = (xt, i * F)
                next_chunk += 1
                t_loaded += n_t

            load_chunk()
            y_t = None
            for t in range(T):
                if t not in x_tiles:
                    load_chunk()
                if next_chunk < len(X_CHUNKS) and t == t_loaded - X_CHUNKS[next_chunk - 1]:
                    load_chunk()  # prefetch one chunk ahead
                xt, off = x_tiles.pop(t)
                xs = xt[:, off:off + F]
                if t % Y_CHUNK == 0:
                    y_t = s_pool.tile([P, Y_CHUNK * F], f32, tag="y")
                ys = y_t[:, (t % Y_CHUNK) * F:(t % Y_CHUNK + 1) * F]
                m = m_pool.tile([P, F], f32, tag="m")
                s = s_pool.tile([P, F], f32, tag="s")
                # m = (v * tau) + x_t
                nc.vector.scalar_tensor_tensor(
                    out=m[:], in0=v[:], scalar=TAU, in1=xs,
                    op0=mybir.AluOpType.mult, op1=mybir.AluOpType.add,
                )
                # v = (m < vth) * m   (hard reset)
                nc.vector.scalar_tensor_tensor(
                    out=v[:], in0=m[:], scalar=V_TH, in1=m[:],
                    op0=mybir.AluOpType.is_lt, op1=mybir.AluOpType.mult,
                )
                # s = Sign(m - 1); y = Sigmoid(1e4*s + 5e3)
                nc.scalar.activation(
                    out=s[:], in_=m[:],
                    func=mybir.ActivationFunctionType.Sign,
                    bias=c_neg1[:], scale=1.0,
                )
                nc.scalar.activation(
                    out=ys, in_=s[:],
                    func=mybir.ActivationFunctionType.Sigmoid,
                    bias=c_bias[:], scale=c_scale[:],
                )
                if t % Y_CHUNK == Y_CHUNK - 1:
                    nc.scalar.dma_start(
                        out=y_r[t - Y_CHUNK + 1:t + 1].rearrange("t p f -> p t f"),
                        in_=y_t[:].rearrange("p (t f) -> p t f", t=Y_CHUNK),
                    )
    nc.compile()
    return nc


def _get_compiled():
    global _compiled
    if _compiled is None:
        _compiled = _build()
    return _compiled


def kernel(x: np.ndarray, _trace: bool = False):
    import concourse.bass_utils as bass_utils

    nc = _get_compiled()
    x = np.ascontiguousarray(x, dtype=np.float32)
    in_maps = [
        {"x": x[:, c * B_SH:(c + 1) * B_SH, :].reshape(T, E)}
        for c in range(N_CORES)
    ]
    res = bass_utils.run_bass_kernel_spmd(
        nc, in_maps, core_ids=list(range(N_CORES)), trace=_trace
    )
    y = np.empty((T, B, N), dtype=np.float32)
    for c in range(N_CORES):
        y[:, c * B_SH:(c + 1) * B_SH, :] = res.results[c]["y"].reshape(T, B_SH, N)
    if _trace:
        return y, res
    return y
